# revision 3
# baseline (speedup 1.0000x reference)
"""CLIP-with-product-quantization kernel for 8 Trainium2 NeuronCores.

Data-parallel over the 8192 rows (1024 rows/core/modality).

Per core:
  - PQ encode: negscore[n,m,k] = 2*dot(v_nm, c_mk) - ||c_mk||^2 via K=17
    fp32 matmuls (17th contraction row folds -||c||^2).  argmin dist ==
    argmax negscore (verified to match the fp32 reference argmin ordering
    including the worst observed 1.9e-6 gap).  Segmented max-reduce gives
    the per-(n,m) max; a fused is_equal builds the one-hot (bf16); PE
    transposes put it k-major; col-tiled one-hot @ codebook matmuls with a
    3-way bf16 codebook split (exact to ~2^-26) gather the quantized
    vectors directly in d-major layout.
  - Text quantized vectors are all-gathered via a DRAM AllGather; each
    core then computes a [1024, 8192] block of
    softmax(100 * img_q @ txt_q^T): fp32 matmuls, row max on DVE,
    Exp on the scalar engine (bias=-100*rowmax, fused row-sum accumulate),
    reciprocal normalize in place, DMA out.
  - Quantize-loss partials (sum of max-scores) are reduced on device;
    the final tiny combine with sum(v^2) happens host-side.
"""

import numpy as np

M, K, D = 32, 256, 512
d = D // M  # 16
N_CORES = 8

_compiled = {}


def _build(nt, tsim=False):
    import concourse.bacc as bacc
    import concourse.mybir as mybir
    import concourse.tile as tile
    from concourse import masks

    dt = mybir.dt
    n_tiles = nt // 128
    ntot = nt * N_CORES
    AX = mybir.AxisListType
    OP = mybir.AluOpType
    AF = mybir.ActivationFunctionType

    nc = bacc.Bacc("TRN2", target_bir_lowering=False, debug=False,
                   enable_asserts=False, num_devices=N_CORES)

    # m = 8*q + g;  partitions 0..16 hold (d rows, then the ones row)
    vTi = nc.dram_tensor("vTi", [17, 32, nt], dt.float32, kind="ExternalInput").ap()
    vTt = nc.dram_tensor("vTt", [17, 32, nt], dt.float32, kind="ExternalInput").ap()
    cbd = nc.dram_tensor("cbd", [17, 8, 4, 256], dt.float32, kind="ExternalInput").ap()
    cbg = nc.dram_tensor("cbg", [128, 2, 3, 8, 4, 16], dt.bfloat16,
                         kind="ExternalInput").ap()

    sim = nc.dram_tensor("sim", [nt, ntot], dt.float32, kind="ExternalOutput").ap()
    msum = nc.dram_tensor("msum", [128, 2], dt.float32, kind="ExternalOutput").ap()

    txtq_loc = nc.dram_tensor("txtq_loc", [128, 4, nt], dt.float32, kind="Internal").ap()
    txtq_all = nc.dram_tensor("txtq_all", [N_CORES * 128 * 4 * nt], dt.float32,
                              kind="Internal", addr_space="Shared").ap()

    vT_views = {
        0: vTi.rearrange("p (q g) n -> p g q n", g=8),
        1: vTt.rearrange("p (q g) n -> p g q n", g=8),
    }

    with tile.TileContext(nc) as tc:
        with tc.tile_pool(name="const", bufs=1) as cpool:
            identb = cpool.tile([128, 128], dt.bfloat16, tag="identb")
            macc = cpool.tile([128, 2], dt.float32, tag="macc")
            qTi = cpool.tile([128, 4, nt], dt.float32, tag="qTi")
            cg = cpool.tile([128, 2, 3, 8, 4, 16], dt.bfloat16, tag="cg")
            nc.sync.dma_start(cg[:], cbg)
            masks.make_identity(nc, identb[:])
            nc.vector.memset(macc[:], 0.0)

            # ---------------- Phase A: PQ encode both modalities ----------------
            compaction_dmas = []
            with tc.tile_pool(name="pq", bufs=2) as pq, \
                 tc.tile_pool(name="pqe", bufs=2) as pqe, \
                 tc.tile_pool(name="pqp", bufs=2, space="PSUM") as pqp, \
                 tc.tile_pool(name="pqg", bufs=2, space="PSUM") as pqg:
                for mod in (0, 1):
                    vTv = vT_views[mod]
                    for g in range(8):
                        cbt = pq.tile([17, 4, 256], dt.float32, tag="cbt")
                        nc.sync.dma_start(cbt[:], cbd[:, g])
                        for t in range(n_tiles):
                            n0 = t * 128
                            vt = pq.tile([17, 4, 128], dt.float32, tag="vt")
                            nc.sync.dma_start(vt[:], vTv[:, g, :, n0:n0 + 128])
                            psd = pqp.tile([128, 4, 256], dt.float32, tag="psd")
                            for q in range(4):
                                nc.tensor.matmul(psd[:, q], vt[:, q, :], cbt[:, q, :],
                                                 start=True, stop=True)
                            mx = pqe.tile([128, 4], dt.float32, tag="mx")
                            nc.vector.tensor_reduce(mx[:], psd[:], axis=AX.X, op=OP.max)
                            eq = pqe.tile([128, 4, 256], dt.bfloat16, tag="eq")
                            for q in range(4):
                                nc.vector.tensor_scalar(
                                    eq[:, q], psd[:, q], mx[:, q:q + 1], None,
                                    op0=OP.is_equal)
                            # loss partial
                            mrs = pqe.tile([128, 1], dt.float32, tag="mrs")
                            nc.vector.tensor_reduce(mrs[:], mx[:], axis=AX.X, op=OP.add)
                            nc.vector.tensor_tensor(macc[:, mod:mod + 1],
                                                    macc[:, mod:mod + 1], mrs[:],
                                                    op=OP.add)
                            # transpose one-hots to k-major
                            eqt = pqe.tile([128, 2, 4, 128], dt.bfloat16, tag="eqt")
                            for q in range(4):
                                for ch in range(2):
                                    ept = pqg.tile([128, 128], dt.bfloat16, tag="ept")
                                    nc.tensor.transpose(
                                        ept[:], eq[:, q, 128 * ch:128 * ch + 128],
                                        identb[:])
                                    nc.vector.tensor_copy(eqt[:, ch, q], ept[:])
                            # gather: col-tiled M=16 matmuls, 6 accumulating per strip
                            psg = pqg.tile([128, 128], dt.float32, tag="psg")
                            for q in range(4):
                                i = 0
                                for ch in range(2):
                                    for sp in range(3):
                                        nc.tensor.matmul(
                                            psg[32 * q:32 * q + 16, :],
                                            cg[:, ch, sp, g, q, :],
                                            eqt[:, ch, q, :],
                                            start=(i == 0), stop=(i == 5),
                                            tile_position=(0, 32 * q))
                                        i += 1
                            qts = pq.tile([128, 128], dt.float32, tag="qts")
                            nc.scalar.copy(qts[:], psg[:])
                            # compaction: strip q rows [32q,32q+16) -> chunk q rows [16g,16g+16)
                            for q in range(4):
                                if mod == 0:
                                    nc.sync.dma_start(
                                        qTi[16 * g:16 * g + 16, q, n0:n0 + 128],
                                        qts[32 * q:32 * q + 16, :])
                                else:
                                    dma = nc.sync.dma_start(
                                        txtq_loc[16 * g:16 * g + 16, q, n0:n0 + 128],
                                        qts[32 * q:32 * q + 16, :])
                                    compaction_dmas.append(dma)
            nc.sync.dma_start(msum, macc[:])

            # ---------------- Phase B: all-gather text q ----------------
            if tsim:
                # timing-model variant: stand in for the collective with a
                # same-size local DRAM round-trip
                cc_inst = nc.sync.dma_start(
                    txtq_all.rearrange("(c x) -> c x", c=N_CORES)[0].unsqueeze(0),
                    txtq_loc.flatten().unsqueeze(0))
            else:
                cc_inst = nc.gpsimd.collective_compute(
                    "AllGather", mybir.AluOpType.bypass,
                    replica_groups=[list(range(N_CORES))],
                    ins=[txtq_loc.flatten()], outs=[txtq_all])
            for dma in compaction_dmas:
                tile.add_dep_helper(cc_inst.ins, dma.ins, sync=True,
                                    reason="allgather after txtq writes")

            txtq = cpool.tile([128, 4, N_CORES, nt], dt.float32, tag="txtq")
            tq_view = txtq_all.rearrange("(c p k n) -> c p k n",
                                         c=N_CORES, p=128, k=4)
            for cc in range(N_CORES):
                dma = nc.sync.dma_start(txtq[:, :, cc, :], tq_view[cc])
                tile.add_dep_helper(dma.ins, cc_inst.ins, sync=True,
                                    reason="readback after allgather")

            # ---------------- Phase C: similarity + softmax ----------------
            with tc.tile_pool(name="sm", bufs=1) as sm, \
                 tc.tile_pool(name="sms", bufs=2) as sms, \
                 tc.tile_pool(name="smp", bufs=8, space="PSUM") as smp:
                tqf = txtq[:].rearrange("p k c n -> p k (c n)")
                for t in range(n_tiles):
                    n0 = t * 128
                    logits = sm.tile([128, ntot], dt.float32, tag="logits")
                    for b in range(ntot // 512):
                        pss = smp.tile([128, 512], dt.float32, tag="pss")
                        for dc in range(4):
                            nc.tensor.matmul(pss[:],
                                             qTi[:, dc, n0:n0 + 128],
                                             tqf[:, dc, 512 * b:512 * b + 512],
                                             start=(dc == 0), stop=(dc == 3))
                        nc.scalar.copy(logits[:, 512 * b:512 * b + 512], pss[:])
                    rmax = sms.tile([128, 1], dt.float32, tag="rmax")
                    nc.vector.tensor_reduce(rmax[:], logits[:], axis=AX.X, op=OP.max)
                    nbias = sms.tile([128, 1], dt.float32, tag="nbias")
                    nc.vector.tensor_scalar(nbias[:], rmax[:], -100.0, None,
                                            op0=OP.mult)
                    es = sms.tile([128, 2], dt.float32, tag="es")
                    half = ntot // 2
                    for h in range(2):
                        nc.scalar.activation(
                            logits[:, h * half:(h + 1) * half],
                            logits[:, h * half:(h + 1) * half],
                            AF.Exp, bias=nbias[:], scale=100.0,
                            accum_out=es[:, h:h + 1])
                    esum = sms.tile([128, 1], dt.float32, tag="esum")
                    nc.vector.tensor_tensor(esum[:], es[:, 0:1], es[:, 1:2], op=OP.add)
                    recip = sms.tile([128, 1], dt.float32, tag="recip")
                    nc.vector.reciprocal(recip[:], esum[:])
                    nc.scalar.activation(logits[:], logits[:], AF.Copy,
                                         bias=0.0, scale=recip[:])
                    nc.sync.dma_start(sim[n0:n0 + 128, :], logits[:])

    nc.compile()
    return nc


def _host_prep(img, txt, cen):
    """Build per-core input maps (m = 8*q + g layout)."""
    import ml_dtypes
    bf = ml_dtypes.bfloat16

    cn = np.sum(cen.astype(np.float32) * cen, axis=2)  # [M, K]
    cbd = np.zeros((17, 8, 4, 256), np.float32)
    for g in range(8):
        for q in range(4):
            m_ = 8 * q + g
            cbd[0:16, g, q, :] = (2.0 * cen[m_]).T
            cbd[16, g, q, :] = -cn[m_]

    h = cen.astype(bf).astype(np.float32)
    m1 = (cen - h).astype(bf).astype(np.float32)
    l1 = (cen - h - m1).astype(bf)
    splits = [h.astype(bf), m1.astype(bf), l1]
    cbg = np.zeros((128, 2, 3, 8, 4, 16), bf)
    for sp in range(3):
        s = splits[sp].astype(np.float32)
        for g in range(8):
            for q in range(4):
                m_ = 8 * q + g
                for ch in range(2):
                    cbg[:, ch, sp, g, q, :] = s[m_, 128 * ch:128 * ch + 128].astype(bf)

    def vt_aug(vec_shard):
        nt = vec_shard.shape[0]
        v = vec_shard.reshape(nt, M, d)
        out = np.zeros((17, 32, nt), np.float32)
        out[0:16] = np.transpose(v, (2, 1, 0))
        out[16] = 1.0
        return out

    nt = img.shape[0] // N_CORES
    in_maps = []
    for c in range(N_CORES):
        sl = slice(c * nt, (c + 1) * nt)
        in_maps.append({
            "vTi": vt_aug(img[sl]),
            "vTt": vt_aug(txt[sl]),
            "cbd": cbd,
            "cbg": cbg,
        })
    return in_maps


def kernel(image_vec, text_vec, centroids):
    from concourse import bass_utils

    image_vec = np.asarray(image_vec, dtype=np.float32)
    text_vec = np.asarray(text_vec, dtype=np.float32)
    centroids = np.asarray(centroids, dtype=np.float32)
    n = image_vec.shape[0]
    nt = n // N_CORES

    if nt not in _compiled:
        _compiled[nt] = _build(nt)
    nc = _compiled[nt]

    in_maps = _host_prep(image_vec, text_vec, centroids)
    res = bass_utils.run_bass_kernel_spmd(nc, in_maps, core_ids=list(range(N_CORES)))

    sim = np.concatenate([res.results[c]["sim"] for c in range(N_CORES)], axis=0)

    # sum_{n,m} dist_min = sum(v^2) - sum(max-scores)
    msum_i = sum(float(res.results[c]["msum"][:, 0].sum()) for c in range(N_CORES))
    msum_t = sum(float(res.results[c]["msum"][:, 1].sum()) for c in range(N_CORES))
    vsq_i = float(np.sum(image_vec.astype(np.float64) ** 2))
    vsq_t = float(np.sum(text_vec.astype(np.float64) ** 2))
    loss = 2.0 * ((vsq_i - msum_i) + (vsq_t - msum_t)) / n
    return sim, np.float32(loss)
